# revision 4
# baseline (speedup 1.0000x reference)
"""HSCD GNN message passing on 8 Trainium2 NeuronCores.

Strategy (dst-node sharding):
  - Nodes padded to NPAD=230400 = 8 * 28800; core c owns dst rows
    [c*28800, (c+1)*28800) = 225 windows of 128 nodes.
  - Per layer, host sorts that core's edges by dst window, pads every window
    to B*128 edges, and emits per-block columns: src row ids (gather offsets),
    dst_rel in [0,128) (or -1 for padding), and dis[src] values.
  - Device, per 128-edge block: indirect-DMA gather of 128 rows [128,64] f16
    from the full previous-layer table; one DVE tensor_scalar builds the
    scaled one-hot M[p,j] = (iota[j]==dst_rel[p]) * dis_src[p]; one PE matmul
    accumulates M.T @ msg into the window's PSUM [128,64] f32.
  - Window flush: h = PSUM * dis_dst; row-normalize (Square+accum, sqrt(+eps),
    reciprocal); out = h/||h|| + x_prev; acc += out; write f32 shard + f16
    AllGather input.
  - AllGather (f16) publishes each layer's full table for the next layer's
    gathers (needed after ubg, view, cart only).
  - Output: acc/5 per shard; host concatenates shards.
"""
import numpy as np
import concourse.bacc as bacc
import concourse.bass as bass
import concourse.mybir as mybir
import concourse.tile as tile
from concourse import bass_utils

NC = 8
P = 128
D = 64
N = 230002
NPAD = 230400
S = NPAD // NC          # 28800 rows per core
NW = S // P             # 225 windows per core

f32 = mybir.dt.float32
f16 = mybir.dt.float16
i32 = mybir.dt.int32

_NC_CACHE = {}

# layer name -> (gather table, residual-shard source, publishes table?)
LAYERS = [
    ("ubg", "x0", "x0", True),
    ("view", "ubg", "ubg", True),
    ("cart", "ubg", "ubg", True),
    ("vbuy", "view", "view", False),
    ("cbuy", "cart", "cart", False),
]


def _preprocess_layer(edge, dis):
    """edge [2,E] int64 -> per-core (offs[P,NW*B], rel[P,NW*B], dsrc[P,NW*B]), B."""
    src = np.asarray(edge[0]).astype(np.int64)
    dst = np.asarray(edge[1]).astype(np.int64)
    order = np.argsort(dst, kind="stable")
    src_s = src[order].astype(np.int32)
    dst_s = dst[order].astype(np.int32)
    bounds = np.searchsorted(dst_s, np.arange(NC + 1) * S)
    cores = []
    B = 1
    for c in range(NC):
        lo, hi = bounds[c], bounds[c + 1]
        w_ids = (dst_s[lo:hi] - c * S) // P
        cnt = np.bincount(w_ids, minlength=NW)
        if cnt.size:
            B = max(B, int(np.ceil(cnt.max() / P)))
        cores.append((lo, hi, w_ids, cnt))
    cap = B * P
    out = []
    for c in range(NC):
        lo, hi, w_ids, cnt = cores[c]
        starts = np.zeros(NW, np.int64)
        np.cumsum(cnt[:-1], out=starts[1:])
        pos = np.arange(hi - lo) - starts[w_ids]
        src_pad = np.zeros((NW, cap), np.int32)
        rel_pad = np.full((NW, cap), -1.0, np.float32)
        dsc_pad = np.zeros((NW, cap), np.float32)
        sl_src = src_s[lo:hi]
        src_pad[w_ids, pos] = sl_src
        rel_pad[w_ids, pos] = (dst_s[lo:hi] - c * S) % P
        dsc_pad[w_ids, pos] = dis[sl_src]
        out.append((
            np.ascontiguousarray(src_pad.reshape(NW * B, P).T),
            np.ascontiguousarray(rel_pad.reshape(NW * B, P).T),
            np.ascontiguousarray(dsc_pad.reshape(NW * B, P).T),
        ))
    return out, B


def _build(Bs):
    """Compile the SPMD kernel for per-layer block counts Bs (dict name->B)."""
    nc = bacc.Bacc("TRN2", target_bir_lowering=False, debug=False, num_devices=NC)

    xfull0 = nc.dram_tensor("xfull0", [NPAD, D], f16, kind="ExternalInput")
    xsh0 = nc.dram_tensor("xsh0", [S, D], f32, kind="ExternalInput")
    ins = {}
    for name, _, _, _ in LAYERS:
        nb = NW * Bs[name]
        ins[name] = dict(
            offs=nc.dram_tensor(f"offs_{name}", [P, nb], i32, kind="ExternalInput"),
            rel=nc.dram_tensor(f"rel_{name}", [P, nb], f32, kind="ExternalInput"),
            dsc=nc.dram_tensor(f"dsc_{name}", [P, nb], f32, kind="ExternalInput"),
            ddst=nc.dram_tensor(f"ddst_{name}", [P, NW], f32, kind="ExternalInput"),
        )
    out_shard = nc.dram_tensor("out_shard", [S, D], f32, kind="ExternalOutput")

    xsh = {"x0": xsh0}
    xfull = {"x0": xfull0}
    agin = {}
    for name, _, _, pub in LAYERS:
        if pub:
            xsh[name] = nc.dram_tensor(f"xsh_{name}", [S, D], f32, kind="Internal")
            agin[name] = nc.dram_tensor(f"agin_{name}", [S, D], f16, kind="Internal")
            xfull[name] = nc.dram_tensor(f"xfull_{name}", [NPAD, D], f16,
                                         kind="Internal", addr_space="Shared")

    with tile.TileContext(nc) as tc:
        with (
            tc.tile_pool(name="io", bufs=2) as io,
            tc.tile_pool(name="blk", bufs=16) as sb,
            tc.tile_pool(name="fl", bufs=6) as fl,
            tc.tile_pool(name="accp", bufs=1) as accp,
            tc.tile_pool(name="psum", bufs=6, space="PSUM") as ps,
        ):
            iota_t = accp.tile([P, P], f16)
            nc.gpsimd.iota(iota_t[:], pattern=[[1, P]], base=0, channel_multiplier=0,
                           allow_small_or_imprecise_dtypes=True)
            acc_t = accp.tile([P, NW * D], f32)
            nc.vector.memset(acc_t[:], 0.0)

            for name, gsrc, prev, pub in LAYERS:
                B = Bs[name]
                nb = NW * B
                off_t = io.tile([P, nb], i32, tag="off")
                dr_t = io.tile([P, nb], f32, tag="dr")
                dv_t = io.tile([P, nb], f32, tag="dv")
                dd_t = io.tile([P, NW], f32, tag="dd")
                nc.sync.dma_start(out=off_t[:], in_=ins[name]["offs"][:, :])
                nc.sync.dma_start(out=dr_t[:], in_=ins[name]["rel"][:, :])
                nc.sync.dma_start(out=dv_t[:], in_=ins[name]["dsc"][:, :])
                nc.sync.dma_start(out=dd_t[:], in_=ins[name]["ddst"][:, :])
                table = xfull[gsrc]
                for w in range(NW):
                    acc_ps = ps.tile([P, D], f32, space="PSUM", tag="acc")
                    for b in range(B):
                        blk = w * B + b
                        g = sb.tile([P, D], f16, tag="g")
                        nc.gpsimd.indirect_dma_start(
                            out=g[:], out_offset=None, in_=table[:],
                            in_offset=bass.IndirectOffsetOnAxis(
                                ap=off_t[:, blk:blk + 1], axis=0))
                        m_t = sb.tile([P, P], f16, tag="m")
                        nc.vector.tensor_scalar(
                            out=m_t[:], in0=iota_t[:],
                            scalar1=dr_t[:, blk:blk + 1],
                            scalar2=dv_t[:, blk:blk + 1],
                            op0=mybir.AluOpType.is_equal,
                            op1=mybir.AluOpType.mult)
                        nc.tensor.matmul(out=acc_ps[:], lhsT=m_t[:], rhs=g[:],
                                         start=(b == 0), stop=(b == B - 1))
                    h_t = fl.tile([P, D], f32, tag="h")
                    nc.scalar.activation(out=h_t[:], in_=acc_ps[:],
                                         func=mybir.ActivationFunctionType.Copy,
                                         scale=dd_t[:, w:w + 1])
                    sq_t = fl.tile([P, D], f32, tag="sq")
                    ss_t = fl.tile([P, 1], f32, tag="ss")
                    nc.scalar.activation(out=sq_t[:], in_=h_t[:],
                                         func=mybir.ActivationFunctionType.Square,
                                         accum_out=ss_t[:, :1])
                    nc.scalar.sqrt(ss_t[:], ss_t[:])
                    nc.vector.tensor_scalar_max(ss_t[:], ss_t[:], 1e-12)
                    inv_t = fl.tile([P, 1], f32, tag="inv")
                    nc.vector.reciprocal(inv_t[:], ss_t[:])
                    o_t = fl.tile([P, D], f32, tag="o")
                    nc.scalar.activation(out=o_t[:], in_=h_t[:],
                                         func=mybir.ActivationFunctionType.Copy,
                                         scale=inv_t[:, :1])
                    xp_t = fl.tile([P, D], f32, tag="xp")
                    nc.sync.dma_start(out=xp_t[:], in_=xsh[prev][w * P:(w + 1) * P, :])
                    nc.vector.tensor_add(o_t[:], o_t[:], xp_t[:])
                    nc.vector.tensor_add(acc_t[:, w * D:(w + 1) * D],
                                         acc_t[:, w * D:(w + 1) * D], o_t[:])
                    if pub:
                        nc.sync.dma_start(out=xsh[name][w * P:(w + 1) * P, :],
                                          in_=o_t[:])
                        o16_t = fl.tile([P, D], f16, tag="o16")
                        nc.vector.tensor_copy(o16_t[:], o_t[:])
                        nc.sync.dma_start(out=agin[name][w * P:(w + 1) * P, :],
                                          in_=o16_t[:])
                if pub:
                    nc.gpsimd.collective_compute(
                        "AllGather", mybir.AluOpType.bypass,
                        replica_groups=[list(range(NC))],
                        ins=[agin[name][:, :]],
                        outs=[xfull[name][:, :]])

            nc.scalar.activation(out=acc_t[:], in_=acc_t[:],
                                 func=mybir.ActivationFunctionType.Copy,
                                 scale=0.2)
            nc.sync.dma_start(
                out=out_shard.rearrange("(w p) d -> p w d", p=P),
                in_=acc_t[:].rearrange("p (w d) -> p w d", w=NW))
    nc.compile()
    return nc


def kernel(user_table, item_table, edge_ubg, edge_view, edge_cart,
           edge_view_buy, edge_cart_buy):
    x0 = np.concatenate([np.asarray(user_table, np.float32),
                         np.asarray(item_table, np.float32)], axis=0)
    x0p = np.zeros((NPAD, D), np.float32)
    x0p[:N] = x0
    xfull0 = x0p.astype(np.float16)

    edges = dict(ubg=edge_ubg, view=edge_view, cart=edge_cart,
                 vbuy=edge_view_buy, cbuy=edge_cart_buy)
    per_core = {}
    Bs = {}
    ddst = {}
    for name in edges:
        e = np.asarray(edges[name])
        dst = e[1].astype(np.int64)
        deg = np.bincount(dst, minlength=NPAD).astype(np.float64)
        dis = np.where(deg > 0, 1.0 / np.sqrt(np.maximum(deg, 1.0)), 0.0).astype(np.float32)
        per_core[name], Bs[name] = _preprocess_layer(e, dis)
        ddst[name] = dis

    key = tuple(sorted(Bs.items()))
    if key not in _NC_CACHE:
        _NC_CACHE[key] = _build(Bs)
    nc = _NC_CACHE[key]

    in_maps = []
    for c in range(NC):
        m = dict(xfull0=xfull0, xsh0=np.ascontiguousarray(x0p[c * S:(c + 1) * S]))
        for name in edges:
            offs, rel, dsc = per_core[name][c]
            m[f"offs_{name}"] = offs
            m[f"rel_{name}"] = rel
            m[f"dsc_{name}"] = dsc
            m[f"ddst_{name}"] = np.ascontiguousarray(
                ddst[name][c * S:(c + 1) * S].reshape(NW, P).T)
        in_maps.append(m)

    res = bass_utils.run_bass_kernel_spmd(nc, in_maps, core_ids=list(range(NC)))
    out = np.concatenate([res.results[c]["out_shard"] for c in range(NC)], axis=0)
    return out[:N].astype(np.float32)
